# revision 33
# baseline (speedup 1.0000x reference)
"""Binary-tree gated-expert MoE (root -> 2 mid -> 4 leaf experts) on 8 trn2 cores.

Strategy: expert-parallel dispatch by leaf index. Tokens are sorted by their
2-bit routing path (leaf = 2*bit0 + bit1) and the sorted order is cut into 8
equal contiguous chunks of C = ceil(B/8) tokens, one per NeuronCore — perfect
load balance. Each core applies ONE expert per layer (the majority expert of
its chunk); the handful of tokens in a chunk that straddle a leaf boundary
(~tens of tokens) get wrong-expert results on the device and are recomputed
exactly (fp32) on the host afterwards. This keeps the SPMD program free of
per-range weight switching.

A core runs 3 chained dense [C,2048]x[2048,2048] layers (root W0, mid
W1[bit0], leaf W2[leaf]) with relu+bias, entirely on-chip. Activations stay
transposed ([D, tokens] feature-major) so each layer's matmul output (PSUM
[fout, tok]) is directly the next layer's rhs. Matmuls run in fp16 (same
TensorE rate as bf16) with fp32 PSUM accumulation; weights stream from HBM as
pre-tiled [16, 128, 2048] stripes used as the stationary operand.

Head scheduling: the PE clock (HAM) sits at 1.2 GHz until ~3.4us of sustained
busy, and the first DMA can't land before the ~7us boot preamble ends. A
block of throwaway warm-up matmuls keeps the PE busy from boot to first-data
so the real stream runs at 2.4 GHz from its first instruction. x chunks
alternate between the two hardware DGE rings (sync + scalar), interleaved
with the first weight stripe's k-slices in exactly the order the layer-0
k-outer loop consumes them.
"""

import numpy as np
from contextlib import ExitStack

import concourse.bass as bass
from concourse import bacc, mybir, tile
from concourse.bass_utils import run_bass_kernel_spmd

D = 2048
PT = 128           # partition tile
KT = D // PT       # 16 contraction tiles per layer
MT = D // PT       # 16 output-feature tiles per layer
N_CORES = 8

F32 = mybir.dt.float32
F16 = mybir.dt.float16
NP_F16 = np.float16

NWARM = 16         # HAM warm-up matmuls issued before real work
WN = 256           # free-dim of each warm-up matmul

# cache of compiled bass programs keyed by (C, TN, NT)
_compiled = {}
# stash of the last run's results so a harness can inspect exec_time_ns
last_results = None


def _prep_weight(W):
    """[D, D] -> [MT, 128, D] fp16: stripe m holds W[:, m*128:(m+1)*128]
    rearranged so partition p = contraction row within k-chunk, and the free
    dim is (k, fout-col) — i.e. out[m, p, k*128 + c] = W[k*128 + p, m*128 + c].
    Each [128, 2048] stripe then DMAs contiguously into SBUF and its k-th
    [128, 128] column block is exactly the lhsT (stationary) matmul operand."""
    W4 = W.reshape(KT, PT, MT, PT)
    return np.ascontiguousarray(
        W4.transpose(2, 1, 0, 3).reshape(MT, PT, D).astype(NP_F16)
    )


def _prep_bias(parts):
    """list of [D] biases -> [128, len*MT] f32 where column i*MT + m holds
    parts[i][m*128 : (m+1)*128] along partitions."""
    cols = [b.reshape(MT, PT).T for b in parts]
    return np.ascontiguousarray(np.concatenate(cols, axis=1).astype(np.float32))


def _build(C, TN, NT):
    """Build + compile the 3-layer SPMD program for per-core capacity C.

    Layer-0 matmuls must consume the 16 k-chunks of the input as they stream
    in, so the m loop runs in pairs: each pair's k-loop trickles behind the
    input DMA instead of one m-tile waiting for the entire input."""
    nc = bacc.Bacc(
        "TRN2",
        target_bir_lowering=False,
        debug=False,
        enable_asserts=False,
        num_devices=N_CORES,
    )
    xT = nc.dram_tensor("xT", [D, C], F16, kind="ExternalInput").ap()
    w0 = nc.dram_tensor("w0", [MT, PT, D], F16, kind="ExternalInput").ap()
    w1 = nc.dram_tensor("w1", [MT, PT, D], F16, kind="ExternalInput").ap()
    w2 = nc.dram_tensor("w2", [MT, PT, D], F16, kind="ExternalInput").ap()
    bias = nc.dram_tensor("bias", [PT, 3 * MT], F32, kind="ExternalInput").ap()
    yT = nc.dram_tensor("yT", [D, C], F16, kind="ExternalOutput").ap()

    with tile.TileContext(nc) as tc, ExitStack() as ctx:
        wpool = ctx.enter_context(tc.tile_pool(name="w", bufs=6))
        hpool = ctx.enter_context(tc.tile_pool(name="h", bufs=1))
        ps_bufs = 8 if TN <= 512 else 4
        pspool = ctx.enter_context(tc.tile_pool(name="ps", bufs=ps_bufs, space="PSUM"))
        opool = ctx.enter_context(tc.tile_pool(name="o", bufs=4))
        cpool = ctx.enter_context(tc.tile_pool(name="c", bufs=2))

        # HAM warm-up: PE is free from end of boot preamble (~7us) until the
        # first x chunk lands (~9.5us); fill that window with throwaway
        # matmuls on a memset scratch tile so the real stream starts at
        # 2.4 GHz.
        warm_sb = cpool.tile([PT, max(WN, PT)], F16, tag="warm")
        nc.vector.memset(warm_sb[:], 0.0)
        warm_ps = pspool.tile([PT, TN], F32, tag="ps", name="warmps")
        for _ in range(NWARM):
            nc.tensor.matmul(
                warm_ps[:, 0:WN], warm_sb[:, 0:PT], warm_sb[:, 0:WN],
                start=True, stop=True, skip_group_check=True,
            )

        hA = hpool.tile([PT, KT, C], F16, tag="hA")
        hB = hpool.tile([PT, KT, C], F16, tag="hB")

        # Head DMA choreography. Layer-0's pair-0 k-loop consumes x chunk k
        # and w0-stripe columns [k*128:(k+1)*128] every ~0.9us. x chunks
        # 1,5,9,13 ride the scalar ring interleaved with the w0 k-slices in
        # consumption order; the rest of x rides the sync ring ascending, so
        # neither ring's first transfers block a consumer that's already
        # runnable.
        wts0 = []
        for m in (0, 1):
            wt = wpool.tile([PT, D], F16, tag="wt", name=f"wt0_{m}")
            nc.scalar.dma_start(wt[:, 0:PT], w0[m, :, 0:PT])
            wts0.append(wt)

        def xdma(eng, k):
            eng.dma_start(hA[:, k, :], xT[k * PT : (k + 1) * PT, :])

        # Head budget: x (4.2MB) + the w0 k-slices (0.98MB) are all consumed
        # inside pair-0's ~14us k-loop, and each DMA_DIRECT2D instruction
        # costs ~0.6us of engine issue time — so the head must be both
        # bandwidth-balanced across the two rings AND use few instructions,
        # ordered by first consumption. Scalar: w0 k0-slices, x1, w0 rest in
        # two halves, x5, x9, x13. Sync: everything else, ascending.
        bias_sb = cpool.tile([PT, 3 * MT], F32)
        nc.gpsimd.dma_start(bias_sb[:], bias[:])
        nc.sync.dma_start(hA[:, 0, 0:TN], xT[0:PT, 0:TN])
        if TN < C:
            nc.sync.dma_start(hA[:, 0, TN:C], xT[0:PT, TN:C])
        xdma(nc.scalar, 1)
        xdma(nc.sync, 2)
        for m in (0, 1):
            nc.scalar.dma_start(wts0[m][:, PT : 8 * PT], w0[m, :, PT : 8 * PT])
        for k in (3, 4):
            xdma(nc.sync, k)
        xdma(nc.scalar, 5)
        for k in (6, 7):
            xdma(nc.sync, k)
        for m in (0, 1):
            nc.scalar.dma_start(wts0[m][:, 8 * PT : D], w0[m, :, 8 * PT : D])
        xdma(nc.sync, 8)
        xdma(nc.scalar, 9)
        for k in (10, 11, 12):
            xdma(nc.sync, k)
        xdma(nc.scalar, 13)
        for k in (14, 15):
            xdma(nc.sync, k)

        # relu+bias epilogues all run on DVE: keeping ScalarE activation-free
        # avoids its ACT_TABLE_LOAD, which would sit in front of the scalar
        # ring's first x-chunk DMA during the critical head.
        def relu_bias(out_ap, ps_ap, b_ap):
            nc.vector.tensor_scalar(
                out_ap, ps_ap, b_ap, 0.0,
                mybir.AluOpType.add, mybir.AluOpType.max,
            )

        layers = [(w0, 0, hA, hB), (w1, 1, hB, hA), (w2, 2, hA, None)]
        nstripe = 0
        for w_dram, li, h_in, h_out in layers:
            for mp in range(MT // 2):
                ms = (2 * mp, 2 * mp + 1)
                if li == 0 and mp == 0:
                    wts = wts0
                else:
                    wts = []
                    for m in ms:
                        wt = wpool.tile([PT, D], F16, tag="wt", name=f"wt{li}_{m}")
                        # alternate stripes across the two hardware rings so
                        # neither ring gates the pair cadence
                        eng = nc.sync if nstripe % 2 else nc.scalar
                        nstripe += 1
                        eng.dma_start(wt[:], w_dram[m])
                        wts.append(wt)
                pss = {
                    (m, n): pspool.tile([PT, TN], F32, tag="ps", name=f"ps{li}_{m}_{n}")
                    for m in ms
                    for n in range(NT)
                }

                def epilogue(mi, m, n):
                    b_ap = bias_sb[:, li * MT + m : li * MT + m + 1]
                    if h_out is not None:
                        relu_bias(
                            h_out[:, m, bass.ts(n, TN)], pss[(m, n)][:], b_ap,
                        )
                    else:
                        # final layer: emit f16 (host upcasts) — halves the
                        # write traffic and the last tile's tail DMA
                        ot = opool.tile([PT, TN], F16, tag="ot", name=f"ot{m}_{n}")
                        relu_bias(ot[:], pss[(m, n)][:], b_ap)
                        dma_eng = nc.sync if (n + mi) % 2 == 1 else nc.scalar
                        dma_eng.dma_start(
                            yT[m * PT : (m + 1) * PT, bass.ts(n, TN)], ot[:]
                        )

                if li == 0:
                    # k-outer: consume the streaming input chunks as they land
                    for k in range(KT):
                        for mi, m in enumerate(ms):
                            for n in range(NT):
                                nc.tensor.matmul(
                                    pss[(m, n)][:],
                                    wts[mi][:, k * PT : (k + 1) * PT],
                                    h_in[:, k, bass.ts(n, TN)],
                                    start=(k == 0),
                                    stop=(k == KT - 1),
                                    skip_group_check=True,
                                )
                            if mp == 0 and k <= 2 and mi == 1:
                                # filler matmuls: the k=1,2 chunks + w0
                                # k-slices can't land before ~14us (ring
                                # bandwidth + DMA-completion latency); hold
                                # the PE busy through that window so the HAM
                                # clock stays warm and later k runs at 2.4 GHz
                                for _ in range((16, 8, 4)[k]):
                                    nc.tensor.matmul(
                                        warm_ps[:, 0:PT], warm_sb[:, 0:PT],
                                        warm_sb[:, 0:PT],
                                        start=True, stop=True,
                                        skip_group_check=True,
                                    )
                    for mi, m in enumerate(ms):
                        for n in range(NT):
                            epilogue(mi, m, n)
                else:
                    # inputs resident: k-inner per tile, so each tile's
                    # epilogue (and final-layer out-DMA) fires as soon as its
                    # accumulation completes — the kernel tail drains one
                    # tile, not four
                    for mi, m in enumerate(ms):
                        for n in range(NT):
                            is_last = (
                                li == 2 and mp == MT // 2 - 1
                                and mi == 1 and n == NT - 1
                            )
                            if not is_last:
                                for k in range(KT):
                                    nc.tensor.matmul(
                                        pss[(m, n)][:],
                                        wts[mi][:, k * PT : (k + 1) * PT],
                                        h_in[:, k, bass.ts(n, TN)],
                                        start=(k == 0),
                                        stop=(k == KT - 1),
                                    )
                                epilogue(mi, m, n)
                                continue
                            # very last tile: run it as two column halves so
                            # the first half's epilogue + out-DMA overlap the
                            # second half's accumulation — halves the kernel
                            # tail after the final matmul. start=True only on
                            # the first half (start clears the whole PSUM
                            # bank); the second half's k0 overwrites via the
                            # cleared has_written bits.
                            H = TN // 2
                            b_ap = bias_sb[:, li * MT + m : li * MT + m + 1]
                            for half in (0, 1):
                                s, e = half * H, half * H + H
                                for k in range(KT):
                                    nc.tensor.matmul(
                                        pss[(m, n)][:, s:e],
                                        wts[mi][:, k * PT : (k + 1) * PT],
                                        h_in[:, k, n * TN + s : n * TN + e],
                                        start=(k == 0 and half == 0),
                                        stop=(k == KT - 1),
                                        skip_group_check=True,
                                    )
                                ot = opool.tile(
                                    [PT, H], F16, tag="ot", name=f"otl_{half}"
                                )
                                relu_bias(ot[:], pss[(m, n)][:, s:e], b_ap)
                                dma_eng = nc.scalar if half == 0 else nc.sync
                                dma_eng.dma_start(
                                    yT[m * PT : (m + 1) * PT, n * TN + s : n * TN + e],
                                    ot[:],
                                )
    nc.compile()
    return nc


def kernel(x, W0, b0, W1, b1, W2, b2, path_mask):
    global last_results
    x = np.asarray(x, dtype=np.float32)
    path_mask = np.asarray(path_mask)
    W0, b0, W1, b1, W2, b2 = (
        np.asarray(a, dtype=np.float32) for a in (W0, b0, W1, b1, W2, b2)
    )
    B = x.shape[0]

    bit0 = path_mask[:, 0].astype(np.int64)
    bit1 = path_mask[:, 1].astype(np.int64)
    leaf = 2 * bit0 + bit1
    order = np.argsort(leaf, kind="stable")
    sleaf = leaf[order]

    C = -(-B // N_CORES)
    NT = -(-C // 512)
    TN = -(-C // NT)
    C = NT * TN

    key = (C, TN, NT)
    if key not in _compiled:
        _compiled[key] = _build(C, TN, NT)
    nc = _compiled[key]

    w_prepped = {}
    def wp(tag, W):
        if tag not in w_prepped:
            w_prepped[tag] = _prep_weight(W)
        return w_prepped[tag]

    xb = x.astype(NP_F16)
    in_maps = []
    groups = []      # (token array, core leaf) per core
    for c in range(N_CORES):
        tok = order[c * C : min((c + 1) * C, B)]
        lv = sleaf[c * C : min((c + 1) * C, B)]
        # majority expert of this chunk; straddle tokens fixed on host later
        lcounts = np.bincount(lv, minlength=4)
        l = int(np.argmax(lcounts))
        groups.append((tok, l))
        xTg = np.zeros((D, C), dtype=NP_F16)
        xTg[:, : len(tok)] = xb[tok].T
        in_maps.append(
            {
                "xT": xTg,
                "w0": wp("w0", W0),
                "w1": wp(("w1", l // 2), W1[l // 2]),
                "w2": wp(("w2", l), W2[l]),
                "bias": _prep_bias([b0, b1[l // 2], b2[l]]),
            }
        )

    last_results = run_bass_kernel_spmd(nc, in_maps, core_ids=list(range(N_CORES)))

    y = np.empty((B, D), dtype=np.float32)
    fix_tok = []
    for (tok, l), res in zip(groups, last_results.results):
        if len(tok):
            y[tok] = res["yT"][:, : len(tok)].T.astype(np.float32)
        wrong = tok[leaf[tok] != l]
        if len(wrong):
            fix_tok.append(wrong)

    # exact host recompute for tokens that ran under the wrong expert
    if fix_tok:
        ft = np.concatenate(fix_tok)
        fl = leaf[ft]
        for lv in np.unique(fl):
            t = ft[fl == lv]
            h = np.maximum(x[t] @ W0 + b0, 0)
            h = np.maximum(h @ W1[lv // 2] + b1[lv // 2], 0)
            y[t] = np.maximum(h @ W2[lv] + b2[lv], 0)
    return y


# revision 34
# speedup vs baseline: 1.0038x; 1.0038x over previous
"""Binary-tree gated-expert MoE (root -> 2 mid -> 4 leaf experts) on 8 trn2 cores.

Strategy: expert-parallel dispatch by leaf index. Tokens are sorted by their
2-bit routing path (leaf = 2*bit0 + bit1) and the sorted order is cut into 8
equal contiguous chunks of C = ceil(B/8) tokens, one per NeuronCore — perfect
load balance. Each core applies ONE expert per layer (the majority expert of
its chunk); the handful of tokens in a chunk that straddle a leaf boundary
(~tens of tokens) get wrong-expert results on the device and are recomputed
exactly (fp32) on the host afterwards. This keeps the SPMD program free of
per-range weight switching.

A core runs 3 chained dense [C,2048]x[2048,2048] layers (root W0, mid
W1[bit0], leaf W2[leaf]) with relu+bias, entirely on-chip. Activations stay
transposed ([D, tokens] feature-major) so each layer's matmul output (PSUM
[fout, tok]) is directly the next layer's rhs. Matmuls run in fp16 (same
TensorE rate as bf16) with fp32 PSUM accumulation; weights stream from HBM as
pre-tiled [16, 128, 2048] stripes used as the stationary operand.

Head scheduling: the PE clock (HAM) sits at 1.2 GHz until ~3.4us of sustained
busy, and the first DMA can't land before the ~7us boot preamble ends. A
block of throwaway warm-up matmuls keeps the PE busy from boot to first-data
so the real stream runs at 2.4 GHz from its first instruction. x chunks
alternate between the two hardware DGE rings (sync + scalar), interleaved
with the first weight stripe's k-slices in exactly the order the layer-0
k-outer loop consumes them.
"""

import numpy as np
from contextlib import ExitStack

import concourse.bass as bass
from concourse import bacc, mybir, tile
from concourse.bass_utils import run_bass_kernel_spmd

D = 2048
PT = 128           # partition tile
KT = D // PT       # 16 contraction tiles per layer
MT = D // PT       # 16 output-feature tiles per layer
N_CORES = 8

F32 = mybir.dt.float32
F16 = mybir.dt.float16
NP_F16 = np.float16

NWARM = 16         # HAM warm-up matmuls issued before real work
WN = 256           # free-dim of each warm-up matmul

# cache of compiled bass programs keyed by (C, TN, NT)
_compiled = {}
# stash of the last run's results so a harness can inspect exec_time_ns
last_results = None


def _prep_weight(W):
    """[D, D] -> [MT, 128, D] fp16: stripe m holds W[:, m*128:(m+1)*128]
    rearranged so partition p = contraction row within k-chunk, and the free
    dim is (k, fout-col) — i.e. out[m, p, k*128 + c] = W[k*128 + p, m*128 + c].
    Each [128, 2048] stripe then DMAs contiguously into SBUF and its k-th
    [128, 128] column block is exactly the lhsT (stationary) matmul operand."""
    W4 = W.reshape(KT, PT, MT, PT)
    return np.ascontiguousarray(
        W4.transpose(2, 1, 0, 3).reshape(MT, PT, D).astype(NP_F16)
    )


def _prep_bias(parts):
    """list of [D] biases -> [128, len*MT] f32 where column i*MT + m holds
    parts[i][m*128 : (m+1)*128] along partitions."""
    cols = [b.reshape(MT, PT).T for b in parts]
    return np.ascontiguousarray(np.concatenate(cols, axis=1).astype(np.float32))


def _build(C, TN, NT):
    """Build + compile the 3-layer SPMD program for per-core capacity C.

    Layer-0 matmuls must consume the 16 k-chunks of the input as they stream
    in, so the m loop runs in pairs: each pair's k-loop trickles behind the
    input DMA instead of one m-tile waiting for the entire input."""
    nc = bacc.Bacc(
        "TRN2",
        target_bir_lowering=False,
        debug=False,
        enable_asserts=False,
        num_devices=N_CORES,
    )
    xT = nc.dram_tensor("xT", [D, C], F16, kind="ExternalInput").ap()
    w0 = nc.dram_tensor("w0", [MT, PT, D], F16, kind="ExternalInput").ap()
    w1 = nc.dram_tensor("w1", [MT, PT, D], F16, kind="ExternalInput").ap()
    w2 = nc.dram_tensor("w2", [MT, PT, D], F16, kind="ExternalInput").ap()
    bias = nc.dram_tensor("bias", [PT, 3 * MT], F32, kind="ExternalInput").ap()
    yT = nc.dram_tensor("yT", [D, C], F16, kind="ExternalOutput").ap()

    with tile.TileContext(nc) as tc, ExitStack() as ctx:
        wpool = ctx.enter_context(tc.tile_pool(name="w", bufs=6))
        hpool = ctx.enter_context(tc.tile_pool(name="h", bufs=1))
        ps_bufs = 8 if TN <= 512 else 4
        pspool = ctx.enter_context(tc.tile_pool(name="ps", bufs=ps_bufs, space="PSUM"))
        opool = ctx.enter_context(tc.tile_pool(name="o", bufs=4))
        cpool = ctx.enter_context(tc.tile_pool(name="c", bufs=2))

        # HAM warm-up: PE is free from end of boot preamble (~7us) until the
        # first x chunk lands (~9.5us); fill that window with throwaway
        # matmuls on a memset scratch tile so the real stream starts at
        # 2.4 GHz.
        warm_sb = cpool.tile([PT, max(WN, PT)], F16, tag="warm")
        nc.vector.memset(warm_sb[:], 0.0)
        warm_ps = pspool.tile([PT, TN], F32, tag="ps", name="warmps")
        for _ in range(NWARM):
            nc.tensor.matmul(
                warm_ps[:, 0:WN], warm_sb[:, 0:PT], warm_sb[:, 0:WN],
                start=True, stop=True, skip_group_check=True,
            )

        hA = hpool.tile([PT, KT, C], F16, tag="hA")
        hB = hpool.tile([PT, KT, C], F16, tag="hB")

        # Head DMA choreography. Layer-0's pair-0 k-loop consumes x chunk k
        # and w0-stripe columns [k*128:(k+1)*128] every ~0.9us. x chunks
        # 1,5,9,13 ride the scalar ring interleaved with the w0 k-slices in
        # consumption order; the rest of x rides the sync ring ascending, so
        # neither ring's first transfers block a consumer that's already
        # runnable.
        wts0 = []
        for m in (0, 1):
            wt = wpool.tile([PT, D], F16, tag="wt", name=f"wt0_{m}")
            nc.scalar.dma_start(wt[:, 0:PT], w0[m, :, 0:PT])
            wts0.append(wt)

        def xdma(eng, k):
            eng.dma_start(hA[:, k, :], xT[k * PT : (k + 1) * PT, :])

        # Head budget: x (4.2MB) + the w0 k-slices (0.98MB) are all consumed
        # inside pair-0's ~14us k-loop, and each DMA_DIRECT2D instruction
        # costs ~0.6us of engine issue time — so the head must be both
        # bandwidth-balanced across the two rings AND use few instructions,
        # ordered by first consumption. Scalar: w0 k0-slices, x1, w0 rest in
        # two halves, x5, x9, x13. Sync: everything else, ascending.
        bias_sb = cpool.tile([PT, 3 * MT], F32)
        nc.gpsimd.dma_start(bias_sb[:], bias[:])
        nc.sync.dma_start(hA[:, 0, 0:TN], xT[0:PT, 0:TN])
        if TN < C:
            nc.sync.dma_start(hA[:, 0, TN:C], xT[0:PT, TN:C])
        xdma(nc.scalar, 1)
        xdma(nc.sync, 2)
        for m in (0, 1):
            nc.scalar.dma_start(wts0[m][:, PT : 8 * PT], w0[m, :, PT : 8 * PT])
        for k in (3, 4):
            xdma(nc.sync, k)
        xdma(nc.scalar, 5)
        for k in (6, 7):
            xdma(nc.sync, k)
        for m in (0, 1):
            nc.scalar.dma_start(wts0[m][:, 8 * PT : D], w0[m, :, 8 * PT : D])
        xdma(nc.sync, 8)
        xdma(nc.scalar, 9)
        for k in (10, 11, 12):
            xdma(nc.sync, k)
        xdma(nc.scalar, 13)
        for k in (14, 15):
            xdma(nc.sync, k)

        # relu+bias epilogues all run on DVE: keeping ScalarE activation-free
        # avoids its ACT_TABLE_LOAD, which would sit in front of the scalar
        # ring's first x-chunk DMA during the critical head.
        def relu_bias(out_ap, ps_ap, b_ap):
            nc.vector.tensor_scalar(
                out_ap, ps_ap, b_ap, 0.0,
                mybir.AluOpType.add, mybir.AluOpType.max,
            )

        layers = [(w0, 0, hA, hB), (w1, 1, hB, hA), (w2, 2, hA, None)]
        nstripe = 0
        for w_dram, li, h_in, h_out in layers:
            for mp in range(MT // 2):
                ms = (2 * mp, 2 * mp + 1)
                if li == 0 and mp == 0:
                    wts = wts0
                else:
                    wts = []
                    for m in ms:
                        wt = wpool.tile([PT, D], F16, tag="wt", name=f"wt{li}_{m}")
                        # alternate stripes across the two hardware rings so
                        # neither ring gates the pair cadence
                        eng = nc.sync if nstripe % 2 else nc.scalar
                        nstripe += 1
                        eng.dma_start(wt[:], w_dram[m])
                        wts.append(wt)
                pss = {
                    (m, n): pspool.tile([PT, TN], F32, tag="ps", name=f"ps{li}_{m}_{n}")
                    for m in ms
                    for n in range(NT)
                }

                def epilogue(mi, m, n):
                    b_ap = bias_sb[:, li * MT + m : li * MT + m + 1]
                    if h_out is not None:
                        relu_bias(
                            h_out[:, m, bass.ts(n, TN)], pss[(m, n)][:], b_ap,
                        )
                    else:
                        # final layer: emit f16 (host upcasts) — halves the
                        # write traffic and the last tile's tail DMA
                        ot = opool.tile([PT, TN], F16, tag="ot", name=f"ot{m}_{n}")
                        relu_bias(ot[:], pss[(m, n)][:], b_ap)
                        dma_eng = nc.sync if (n + mi) % 2 == 1 else nc.scalar
                        dma_eng.dma_start(
                            yT[m * PT : (m + 1) * PT, bass.ts(n, TN)], ot[:]
                        )

                if li == 0:
                    # k-outer: consume the streaming input chunks as they land
                    for k in range(KT):
                        for mi, m in enumerate(ms):
                            for n in range(NT):
                                nc.tensor.matmul(
                                    pss[(m, n)][:],
                                    wts[mi][:, k * PT : (k + 1) * PT],
                                    h_in[:, k, bass.ts(n, TN)],
                                    start=(k == 0),
                                    stop=(k == KT - 1),
                                    skip_group_check=True,
                                )
                            if mp == 0 and k <= 2 and mi == 1:
                                # filler matmuls: the k=1,2 chunks + w0
                                # k-slices can't land before ~14.5us (ring
                                # bandwidth + DMA-completion latency); pad
                                # the wait with short matmuls (~53ns each
                                # cold) so the HAM clock stays warm and later
                                # k runs at 2.4 GHz. Sized to the median
                                # data-arrival gap — the pad is not
                                # preemptible, so overshoot delays real work.
                                for _ in range((50, 10, 5)[k]):
                                    nc.tensor.matmul(
                                        warm_ps[:, 0:64], warm_sb[:, 0:PT],
                                        warm_sb[:, 0:64],
                                        start=True, stop=True,
                                        skip_group_check=True,
                                    )
                    for mi, m in enumerate(ms):
                        for n in range(NT):
                            epilogue(mi, m, n)
                else:
                    # inputs resident: k-inner per tile, so each tile's
                    # epilogue (and final-layer out-DMA) fires as soon as its
                    # accumulation completes — the kernel tail drains one
                    # tile, not four
                    for mi, m in enumerate(ms):
                        for n in range(NT):
                            is_last = (
                                li == 2 and mp == MT // 2 - 1
                                and mi == 1 and n == NT - 1
                            )
                            if not is_last:
                                for k in range(KT):
                                    nc.tensor.matmul(
                                        pss[(m, n)][:],
                                        wts[mi][:, k * PT : (k + 1) * PT],
                                        h_in[:, k, bass.ts(n, TN)],
                                        start=(k == 0),
                                        stop=(k == KT - 1),
                                    )
                                epilogue(mi, m, n)
                                continue
                            # very last tile: run it as two column halves so
                            # the first half's epilogue + out-DMA overlap the
                            # second half's accumulation — halves the kernel
                            # tail after the final matmul. start=True only on
                            # the first half (start clears the whole PSUM
                            # bank); the second half's k0 overwrites via the
                            # cleared has_written bits.
                            H = TN // 2
                            b_ap = bias_sb[:, li * MT + m : li * MT + m + 1]
                            for half in (0, 1):
                                s, e = half * H, half * H + H
                                for k in range(KT):
                                    nc.tensor.matmul(
                                        pss[(m, n)][:, s:e],
                                        wts[mi][:, k * PT : (k + 1) * PT],
                                        h_in[:, k, n * TN + s : n * TN + e],
                                        start=(k == 0 and half == 0),
                                        stop=(k == KT - 1),
                                        skip_group_check=True,
                                    )
                                ot = opool.tile(
                                    [PT, H], F16, tag="ot", name=f"otl_{half}"
                                )
                                relu_bias(ot[:], pss[(m, n)][:, s:e], b_ap)
                                dma_eng = nc.scalar if half == 0 else nc.sync
                                dma_eng.dma_start(
                                    yT[m * PT : (m + 1) * PT, n * TN + s : n * TN + e],
                                    ot[:],
                                )
    nc.compile()
    return nc


def kernel(x, W0, b0, W1, b1, W2, b2, path_mask):
    global last_results
    x = np.asarray(x, dtype=np.float32)
    path_mask = np.asarray(path_mask)
    W0, b0, W1, b1, W2, b2 = (
        np.asarray(a, dtype=np.float32) for a in (W0, b0, W1, b1, W2, b2)
    )
    B = x.shape[0]

    bit0 = path_mask[:, 0].astype(np.int64)
    bit1 = path_mask[:, 1].astype(np.int64)
    leaf = 2 * bit0 + bit1
    order = np.argsort(leaf, kind="stable")
    sleaf = leaf[order]

    C = -(-B // N_CORES)
    NT = -(-C // 512)
    TN = -(-C // NT)
    C = NT * TN

    key = (C, TN, NT)
    if key not in _compiled:
        _compiled[key] = _build(C, TN, NT)
    nc = _compiled[key]

    w_prepped = {}
    def wp(tag, W):
        if tag not in w_prepped:
            w_prepped[tag] = _prep_weight(W)
        return w_prepped[tag]

    xb = x.astype(NP_F16)
    in_maps = []
    groups = []      # (token array, core leaf) per core
    for c in range(N_CORES):
        tok = order[c * C : min((c + 1) * C, B)]
        lv = sleaf[c * C : min((c + 1) * C, B)]
        # majority expert of this chunk; straddle tokens fixed on host later
        lcounts = np.bincount(lv, minlength=4)
        l = int(np.argmax(lcounts))
        groups.append((tok, l))
        xTg = np.zeros((D, C), dtype=NP_F16)
        xTg[:, : len(tok)] = xb[tok].T
        in_maps.append(
            {
                "xT": xTg,
                "w0": wp("w0", W0),
                "w1": wp(("w1", l // 2), W1[l // 2]),
                "w2": wp(("w2", l), W2[l]),
                "bias": _prep_bias([b0, b1[l // 2], b2[l]]),
            }
        )

    last_results = run_bass_kernel_spmd(nc, in_maps, core_ids=list(range(N_CORES)))

    y = np.empty((B, D), dtype=np.float32)
    fix_tok = []
    for (tok, l), res in zip(groups, last_results.results):
        if len(tok):
            y[tok] = res["yT"][:, : len(tok)].T.astype(np.float32)
        wrong = tok[leaf[tok] != l]
        if len(wrong):
            fix_tok.append(wrong)

    # exact host recompute for tokens that ran under the wrong expert
    if fix_tok:
        ft = np.concatenate(fix_tok)
        fl = leaf[ft]
        for lv in np.unique(fl):
            t = ft[fl == lv]
            h = np.maximum(x[t] @ W0 + b0, 0)
            h = np.maximum(h @ W1[lv // 2] + b1[lv // 2], 0)
            y[t] = np.maximum(h @ W2[lv] + b2[lv], 0)
    return y
